# revision 6
# baseline (speedup 1.0000x reference)
import numpy as np

import concourse.bacc as bacc
import concourse.mybir as mybir
import concourse.tile as tile
from concourse.bass_utils import run_bass_kernel_spmd

N = 4096
D = 1024
NCLS = 64
CORES = 8
R = N // CORES
NCHUNK = 8
NC0 = 4
KT = 9
F32 = mybir.dt.float32
F32R = mybir.dt.float32r
ALU = mybir.AluOpType
ACT = mybir.ActivationFunctionType
AX = mybir.AxisListType

_CACHE = {}


def build_kernel():
    nc = bacc.Bacc("TRN2", target_bir_lowering=False)
    bTc_d = nc.dram_tensor("bTc", [NCHUNK, KT, 128, 512], F32R, kind="ExternalInput")
    rowsT_d = nc.dram_tensor("rowsT", [KT, 128, 512], F32R, kind="ExternalInput")
    out_d = nc.dram_tensor("out", [128, 8], F32, kind="ExternalOutput")

    with tile.TileContext(nc) as tc:
        with (
            tc.tile_pool(name="rows", bufs=1) as rows_pool,
            tc.tile_pool(name="chunks", bufs=2) as chunk_pool,
            tc.tile_pool(name="c1res", bufs=1) as c1_pool,
            tc.tile_pool(name="c2sp", bufs=1) as c2_pool,
            tc.tile_pool(name="psum", bufs=8, space="PSUM") as psum_pool,
            tc.tile_pool(name="scratch", bufs=2) as scratch_pool,
            tc.tile_pool(name="stats", bufs=1) as stats_pool,
        ):
            rowsT_sb = rows_pool.tile([128, KT, 512], F32R)
            nc.sync.dma_start(rowsT_sb[:], rowsT_d.ap().rearrange("k p f -> p k f"))

            bias3 = stats_pool.tile([128, 1], F32)
            nc.vector.memset(bias3, 3.0)
            bias_p = stats_pool.tile([128, 1], F32)
            nc.vector.memset(bias_p, -121.0)
            bias_n = stats_pool.tile([128, 1], F32)
            nc.vector.memset(bias_n, -140.0)

            c2s = [c2_pool.tile([128, N], F32, name=f"c2s_{m}") for m in range(4)]
            mins = stats_pool.tile([128, 4, NCHUNK], F32)
            maxs = stats_pool.tile([128, 4, NCHUNK], F32)
            pos_parts = stats_pool.tile([128, 4, NCHUNK], F32)
            neg_parts = stats_pool.tile([128, 4, NCHUNK], F32)
            tp = stats_pool.tile([128, 4], F32)
            tn = stats_pool.tile([128, 4], F32)

            c1_tiles = []
            for n in range(NC0, NCHUNK):
                ct = c1_pool.tile([128, KT, 512], F32R, name=f"c1_{n}")
                c1_tiles.append(ct)

            def dma_chunk(dst, n):
                for g in range(3):
                    ks = slice(3 * g, 3 * g + 3)
                    nc.sync.dma_start(
                        dst[:, ks, :],
                        bTc_d.ap()[n, ks].rearrange("k p f -> p k f"),
                    )

            def mm_block(ps, chunk_t, m):
                for k in range(KT):
                    nc.tensor.matmul(
                        ps[:],
                        lhsT=rowsT_sb[:, k, 128 * m : 128 * (m + 1)],
                        rhs=chunk_t[:, k, :],
                        start=(k == 0),
                        stop=(k == KT - 1),
                    )

            def evac_and_mine(ps, m, n):
                seg = c2s[m][:, 512 * n : 512 * (n + 1)]
                nc.scalar.activation(
                    out=seg, in_=ps[:], func=ACT.Identity, bias=bias3[:], scale=1.0
                )
                nc.vector.tensor_reduce(
                    mins[:, m, n : n + 1], seg, axis=AX.X, op=ALU.min
                )
                nc.vector.tensor_reduce(
                    maxs[:, m, n : n + 1], seg, axis=AX.X, op=ALU.max
                )

            for n in range(NC0):
                chunk = chunk_pool.tile([128, KT, 512], F32R, tag="chunk", name="chunk")
                dma_chunk(chunk, n)
                for m in range(4):
                    ps = psum_pool.tile([128, 512], F32, tag="ps", name="ps")
                    mm_block(ps, chunk, m)
                    evac_and_mine(ps, m, n)

            for i, n in enumerate(range(NC0, NCHUNK)):
                dma_chunk(c1_tiles[i], n)

            for m in range(4):
                for i, n in enumerate(range(NC0, NCHUNK)):
                    ps = psum_pool.tile([128, 512], F32, tag="ps", name="ps")
                    mm_block(ps, c1_tiles[i], m)
                    evac_and_mine(ps, m, n)

                minall = stats_pool.tile([128, 1], F32, name=f"minall_{m}")
                maxall = stats_pool.tile([128, 1], F32, name=f"maxall_{m}")
                nc.vector.tensor_reduce(minall[:], mins[:, m, :], axis=AX.X, op=ALU.min)
                nc.vector.tensor_reduce(maxall[:], maxs[:, m, :], axis=AX.X, op=ALU.max)
                nc.vector.tensor_scalar_add(tp[:, m : m + 1], maxall[:], -63.9)
                nc.vector.tensor_scalar_add(tn[:, m : m + 1], minall[:], 63.9)
                for n in range(NCHUNK):
                    seg = c2s[m][:, 512 * n : 512 * (n + 1)]
                    na = scratch_pool.tile([128, 512], F32, tag="na", name="na")
                    nc.vector.scalar_tensor_tensor(
                        out=na[:], in0=seg, scalar=tn[:, m : m + 1], in1=seg,
                        op0=ALU.is_gt, op1=ALU.mult,
                    )
                    nc.scalar.activation(
                        out=na[:], in_=na[:], func=ACT.Exp,
                        bias=bias_n[:], scale=40.0,
                        accum_out=neg_parts[:, m, n : n + 1],
                    )
                    pa = scratch_pool.tile([128, 512], F32, tag="pa", name="pa")
                    nc.vector.scalar_tensor_tensor(
                        out=pa[:], in0=seg, scalar=tp[:, m : m + 1], in1=seg,
                        op0=ALU.is_lt, op1=ALU.mult,
                    )
                    nc.scalar.activation(
                        out=pa[:], in_=pa[:], func=ACT.Exp,
                        bias=bias_p[:], scale=-2.0,
                        accum_out=pos_parts[:, m, n : n + 1],
                    )

            pos_sum = stats_pool.tile([128, 4], F32)
            neg_sum = stats_pool.tile([128, 4], F32)
            for m in range(4):
                nc.vector.tensor_reduce(
                    pos_sum[:, m : m + 1], pos_parts[:, m, :], axis=AX.X, op=ALU.add
                )
                nc.vector.tensor_reduce(
                    neg_sum[:, m : m + 1], neg_parts[:, m, :], axis=AX.X, op=ALU.add
                )
            la = stats_pool.tile([128, 4], F32)
            lb = stats_pool.tile([128, 4], F32)
            nc.scalar.activation(out=la[:], in_=pos_sum[:], func=ACT.Ln, bias=1.0)
            nc.scalar.activation(out=lb[:], in_=neg_sum[:], func=ACT.Ln, bias=1.0)
            lb40 = stats_pool.tile([128, 4], F32)
            nc.vector.tensor_scalar_mul(lb40[:], lb[:], 1.0 / 40.0)
            loss_t = stats_pool.tile([128, 4], F32)
            nc.vector.scalar_tensor_tensor(
                out=loss_t[:], in0=la[:], scalar=0.5, in1=lb40[:],
                op0=ALU.mult, op1=ALU.add,
            )
            vpos = stats_pool.tile([128, 4], F32)
            nc.vector.tensor_scalar(vpos[:], pos_sum[:], 0.0, None, ALU.is_gt)
            valid = stats_pool.tile([128, 4], F32)
            nc.vector.scalar_tensor_tensor(
                out=valid[:], in0=neg_sum[:], scalar=0.0, in1=vpos[:],
                op0=ALU.is_gt, op1=ALU.mult,
            )
            outt = stats_pool.tile([128, 8], F32)
            nc.vector.tensor_tensor(outt[:, 0:4], loss_t[:], valid[:], ALU.mult)
            nc.vector.tensor_copy(outt[:, 4:8], valid[:])
            nc.sync.dma_start(out_d.ap(), outt[:])
    nc.finalize()
    return nc


def prep_inputs(batch, labels):
    batch = np.ascontiguousarray(np.asarray(batch, dtype=np.float32))
    labels = np.asarray(labels)
    bT = batch.T
    oh = (labels[None, :] == np.arange(NCLS)[:, None]).astype(np.float32)
    bTc = np.zeros((NCHUNK, KT, 128, 512), np.float32)
    for n in range(NCHUNK):
        cols = slice(512 * n, 512 * (n + 1))
        bTc[n, :8] = bT[:, cols].reshape(8, 128, 512)
        bTc[n, 8, :NCLS] = oh[:, cols]
    in_maps = []
    for c in range(CORES):
        cols = slice(R * c, R * (c + 1))
        rT = np.zeros((KT, 128, 512), np.float32)
        rT[:8] = bT[:, cols].reshape(8, 128, 512)
        rT[8, :NCLS] = -64.0 * oh[:, cols]
        in_maps.append({"bTc": bTc, "rowsT": rT})
    return in_maps


def run(batch, labels, trace=False):
    if "nc" not in _CACHE:
        _CACHE["nc"] = build_kernel()
    in_maps = prep_inputs(batch, labels)
    res = run_bass_kernel_spmd(
        _CACHE["nc"], in_maps, core_ids=list(range(CORES)), trace=trace
    )
    lv = 0.0
    v = 0.0
    for c in range(CORES):
        o = res.results[c]["out"]
        lv += o[:, 0:4].sum(dtype=np.float64)
        v += o[:, 4:8].sum(dtype=np.float64)
    loss = np.float32(lv / max(v, 1.0))
    return loss, res


def kernel(batch, labels):
    loss, _ = run(batch, labels, trace=False)
    return loss


# revision 7
# speedup vs baseline: 1.0899x; 1.0899x over previous
import numpy as np

import concourse.bacc as bacc
import concourse.mybir as mybir
import concourse.tile as tile
from concourse.bass_utils import run_bass_kernel_spmd

N = 4096
D = 1024
NCLS = 64
CORES = 8
R = N // CORES
NCHUNK = 8
NC0 = 5
KT = 9
F32 = mybir.dt.float32
F32R = mybir.dt.float32r
ALU = mybir.AluOpType
ACT = mybir.ActivationFunctionType
AX = mybir.AxisListType

_CACHE = {}


def build_kernel():
    nc = bacc.Bacc("TRN2", target_bir_lowering=False)
    bTc_d = nc.dram_tensor("bTc", [NCHUNK, KT, 128, 512], F32R, kind="ExternalInput")
    rowsT_d = nc.dram_tensor("rowsT", [KT, 128, 512], F32R, kind="ExternalInput")
    out_d = nc.dram_tensor("out", [128, 8], F32, kind="ExternalOutput")

    with tile.TileContext(nc) as tc:
        with (
            tc.tile_pool(name="rows", bufs=1) as rows_pool,
            tc.tile_pool(name="chunks", bufs=2) as chunk_pool,
            tc.tile_pool(name="c1res", bufs=1) as c1_pool,
            tc.tile_pool(name="c2sp", bufs=1) as c2_pool,
            tc.tile_pool(name="psum", bufs=8, space="PSUM") as psum_pool,
            tc.tile_pool(name="scratch", bufs=2) as scratch_pool,
            tc.tile_pool(name="stats", bufs=1) as stats_pool,
        ):
            rowsT_sb = rows_pool.tile([128, KT, 512], F32R)
            nc.sync.dma_start(rowsT_sb[:], rowsT_d.ap().rearrange("k p f -> p k f"))

            bias3 = stats_pool.tile([128, 1], F32)
            nc.vector.memset(bias3, 3.0)
            bias_p = stats_pool.tile([128, 1], F32)
            nc.vector.memset(bias_p, -121.0)
            bias_n = stats_pool.tile([128, 1], F32)
            nc.vector.memset(bias_n, -140.0)

            c2s = [c2_pool.tile([128, N], F32, name=f"c2s_{m}") for m in range(4)]
            mins = stats_pool.tile([128, 4, NCHUNK], F32)
            maxs = stats_pool.tile([128, 4, NCHUNK], F32)
            pos_parts = stats_pool.tile([128, 4, 2], F32)
            neg_parts = stats_pool.tile([128, 4, 2], F32)
            tp = stats_pool.tile([128, 4], F32)
            tn = stats_pool.tile([128, 4], F32)

            c1_tiles = []
            for n in range(NC0, NCHUNK):
                ct = c1_pool.tile([128, KT, 512], F32R, name=f"c1_{n}")
                c1_tiles.append(ct)

            def dma_chunk(dst, n):
                nc.sync.dma_start(
                    dst[:], bTc_d.ap()[n].rearrange("k p f -> p k f")
                )

            def mm_block(ps, chunk_t, m):
                for k in range(KT):
                    nc.tensor.matmul(
                        ps[:],
                        lhsT=rowsT_sb[:, k, 128 * m : 128 * (m + 1)],
                        rhs=chunk_t[:, k, :],
                        start=(k == 0),
                        stop=(k == KT - 1),
                    )

            def evac_and_mine(ps, m, n):
                seg = c2s[m][:, 512 * n : 512 * (n + 1)]
                nc.scalar.activation(
                    out=seg, in_=ps[:], func=ACT.Identity, bias=bias3[:], scale=1.0
                )
                nc.vector.tensor_reduce(
                    mins[:, m, n : n + 1], seg, axis=AX.X, op=ALU.min
                )
                nc.vector.tensor_reduce(
                    maxs[:, m, n : n + 1], seg, axis=AX.X, op=ALU.max
                )

            for n in range(NC0):
                chunk = chunk_pool.tile([128, KT, 512], F32R, tag="chunk", name="chunk")
                dma_chunk(chunk, n)
                for m in range(4):
                    ps = psum_pool.tile([128, 512], F32, tag="ps", name="ps")
                    mm_block(ps, chunk, m)
                    evac_and_mine(ps, m, n)

            for i, n in enumerate(range(NC0, NCHUNK)):
                dma_chunk(c1_tiles[i], n)

            def phase2(m):
                for h in range(2):
                    seg = c2s[m][:, 2048 * h : 2048 * (h + 1)]
                    na = scratch_pool.tile([128, 2048], F32, tag="na", name="na")
                    nc.vector.scalar_tensor_tensor(
                        out=na[:], in0=seg, scalar=tn[:, m : m + 1], in1=seg,
                        op0=ALU.is_gt, op1=ALU.mult,
                    )
                    nc.scalar.activation(
                        out=na[:], in_=na[:], func=ACT.Exp,
                        bias=bias_n[:], scale=40.0,
                        accum_out=neg_parts[:, m, h : h + 1],
                    )
                    nc.vector.scalar_tensor_tensor(
                        out=seg, in0=seg, scalar=tp[:, m : m + 1], in1=seg,
                        op0=ALU.is_lt, op1=ALU.mult,
                    )
                    nc.scalar.activation(
                        out=seg, in_=seg, func=ACT.Exp,
                        bias=bias_p[:], scale=-2.0,
                        accum_out=pos_parts[:, m, h : h + 1],
                    )

            for m in range(4):
                for i, n in enumerate(range(NC0, NCHUNK)):
                    ps = psum_pool.tile([128, 512], F32, tag="ps", name="ps")
                    mm_block(ps, c1_tiles[i], m)
                    evac_and_mine(ps, m, n)

                minall = stats_pool.tile([128, 1], F32, name=f"minall_{m}")
                maxall = stats_pool.tile([128, 1], F32, name=f"maxall_{m}")
                nc.vector.tensor_reduce(minall[:], mins[:, m, :], axis=AX.X, op=ALU.min)
                nc.vector.tensor_reduce(maxall[:], maxs[:, m, :], axis=AX.X, op=ALU.max)
                nc.vector.tensor_scalar_add(tp[:, m : m + 1], maxall[:], -63.9)
                nc.vector.tensor_scalar_add(tn[:, m : m + 1], minall[:], 63.9)
                if m >= 1:
                    phase2(m - 1)
            phase2(3)

            pos_sum = stats_pool.tile([128, 4], F32)
            neg_sum = stats_pool.tile([128, 4], F32)
            for m in range(4):
                nc.vector.tensor_reduce(
                    pos_sum[:, m : m + 1], pos_parts[:, m, :], axis=AX.X, op=ALU.add
                )
                nc.vector.tensor_reduce(
                    neg_sum[:, m : m + 1], neg_parts[:, m, :], axis=AX.X, op=ALU.add
                )
            la = stats_pool.tile([128, 4], F32)
            lb = stats_pool.tile([128, 4], F32)
            nc.scalar.activation(out=la[:], in_=pos_sum[:], func=ACT.Ln, bias=1.0)
            nc.scalar.activation(out=lb[:], in_=neg_sum[:], func=ACT.Ln, bias=1.0)
            lb40 = stats_pool.tile([128, 4], F32)
            nc.vector.tensor_scalar_mul(lb40[:], lb[:], 1.0 / 40.0)
            loss_t = stats_pool.tile([128, 4], F32)
            nc.vector.scalar_tensor_tensor(
                out=loss_t[:], in0=la[:], scalar=0.5, in1=lb40[:],
                op0=ALU.mult, op1=ALU.add,
            )
            vpos = stats_pool.tile([128, 4], F32)
            nc.vector.tensor_scalar(vpos[:], pos_sum[:], 0.0, None, ALU.is_gt)
            valid = stats_pool.tile([128, 4], F32)
            nc.vector.scalar_tensor_tensor(
                out=valid[:], in0=neg_sum[:], scalar=0.0, in1=vpos[:],
                op0=ALU.is_gt, op1=ALU.mult,
            )
            outt = stats_pool.tile([128, 8], F32)
            nc.vector.tensor_tensor(outt[:, 0:4], loss_t[:], valid[:], ALU.mult)
            nc.vector.tensor_copy(outt[:, 4:8], valid[:])
            nc.sync.dma_start(out_d.ap(), outt[:])
    nc.finalize()
    return nc


def prep_inputs(batch, labels):
    batch = np.ascontiguousarray(np.asarray(batch, dtype=np.float32))
    labels = np.asarray(labels)
    bT = batch.T
    oh = (labels[None, :] == np.arange(NCLS)[:, None]).astype(np.float32)
    bTc = np.zeros((NCHUNK, KT, 128, 512), np.float32)
    for n in range(NCHUNK):
        cols = slice(512 * n, 512 * (n + 1))
        bTc[n, :8] = bT[:, cols].reshape(8, 128, 512)
        bTc[n, 8, :NCLS] = oh[:, cols]
    in_maps = []
    for c in range(CORES):
        cols = slice(R * c, R * (c + 1))
        rT = np.zeros((KT, 128, 512), np.float32)
        rT[:8] = bT[:, cols].reshape(8, 128, 512)
        rT[8, :NCLS] = -64.0 * oh[:, cols]
        in_maps.append({"bTc": bTc, "rowsT": rT})
    return in_maps


def run(batch, labels, trace=False):
    if "nc" not in _CACHE:
        _CACHE["nc"] = build_kernel()
    in_maps = prep_inputs(batch, labels)
    res = run_bass_kernel_spmd(
        _CACHE["nc"], in_maps, core_ids=list(range(CORES)), trace=trace
    )
    lv = 0.0
    v = 0.0
    for c in range(CORES):
        o = res.results[c]["out"]
        lv += o[:, 0:4].sum(dtype=np.float64)
        v += o[:, 4:8].sum(dtype=np.float64)
    loss = np.float32(lv / max(v, 1.0))
    return loss, res


def kernel(batch, labels):
    loss, _ = run(batch, labels, trace=False)
    return loss
